# revision 12
# baseline (speedup 1.0000x reference)
"""Distributed sparse-attention head for Trainium2 (8 NeuronCores).

Math (per batch b):
    Q = q Wq^T + bq ; K = k Wk^T + bk ; V = v Wv^T + bv
    num = Q^T K  (contract over sequence S)
    attn = softmax((num + mask)/sqrt(DK), axis=-1)
    out = attn V^T                       # [DQ, S]

Key restructuring (avoids all big on-device transposes):
    num  = Wq G Wk^T + rank-1 bias terms,  G = q^T k   (natural [s,d] layout!)
    out  = diag(1/rowsum(E)) (E Wv) v^T + rank-1 bv term,  E = exp(scores)
The rank-1 bias corrections are folded into a host-precomputed additive
mask tensor. Softmax max-subtraction is skipped (scores bounded ~ +-30,
exp safe in fp32). Sharding: core c -> (batch b=c//2, seq-half h=c%2);
G is AllReduce'd (add) within the 2-core pair that shares a batch.
"""

import sys

sys.path.insert(0, "/opt/trn_rl_repo")

import numpy as np
import ml_dtypes
import concourse.bass as bass
import concourse.mybir as mybir
import concourse.tile as tile
from concourse.bass_utils import run_bass_kernel_spmd
from concourse.vector_clock import ScopedClock

B, S, DIN, DQ, DK = 4, 8192, 512, 512, 512
SH = S // 2  # 4096 seq positions per core
N_CORES = 8
F32 = mybir.dt.float32
F32R = mybir.dt.float32r
BF16 = mybir.dt.bfloat16
AF = mybir.ActivationFunctionType

# mask value (post-scale): exp(-200) == 0 in fp32, comfortably beyond any score
MASK_NEG = -200.0 * np.sqrt(DK)

TRACE = False
TRACE_DIR = None
LAST_RESULTS = None


def _patched_drain_and_barrier(self, tick_clock, wait_clock):
    # This walrus build rejects >1 sync-wait on the kernel-tail Drain
    # ("Too many sync wait commands"). Put the global-clock waits on
    # no-fuse NOPs (one wait each), then emit a clean drain.
    nc = self.nc
    probe = nc.sync.nop(nofuse=True)
    wait_clock.add_sem_waits(probe.ins, ScopedClock({None: tick_clock.global_clock}))
    waits = list(probe.ins.sync_info.on_wait)
    probe.ins.sync_info.on_wait[:] = waits[:1]
    for w in waits[1:]:
        n2 = nc.sync.nop(nofuse=True)
        if n2.ins.sync_info is None:
            n2.ins.sync_info = mybir.SyncInfo(on_wait=[w], on_update=[])
        else:
            n2.ins.sync_info.on_wait[:] = [w]
    nc.sync.drain()
    nc.all_engine_barrier()
    assert self.sems is not None
    popped = nc._tile_sem_poison_stack.pop()
    assert popped is self._sem_poison
    nc.clear_and_free_semaphores(list(self.sems.allocated().values()))
    nc.all_engine_barrier()


tile.TileContext._drain_and_barrier = _patched_drain_and_barrier


def _split_multi_waits(nc, max_waits=1):
    """This walrus build rejects instructions carrying more than one sync
    wait ("Too many sync wait commands"). Hoist extra waits onto NoOp
    instructions spliced immediately before the carrier, same engine —
    semantically identical (engine blocks on the waits either way)."""
    uid = 0
    for fn in nc.m.functions:
        for bb in fn.blocks:
            new_insts = []
            for ins in bb.instructions:
                si = ins.sync_info
                if si is not None and len(si.on_wait) > max_waits:
                    extra = si.on_wait[: len(si.on_wait) - max_waits]
                    keep = si.on_wait[len(si.on_wait) - max_waits :]
                    for w in extra:
                        uid += 1
                        nop = mybir.InstNoOp(
                            name=f"{ins.name}-wsplit{uid}",
                            ins=[],
                            outs=[],
                        )
                        nop.engine = ins.engine
                        nop.sync_info = mybir.SyncInfo(on_wait=[w], on_update=[])
                        nop.bass_nofuse = True
                        new_insts.append(nop)
                    si.on_wait[:] = keep
                new_insts.append(ins)
            bb.instructions[:] = new_insts


_NC_CACHE = None


def _build():
    """Build the SPMD program (identical on all 8 cores)."""
    nc = bass.Bass(target_bir_lowering=False)

    qs = nc.dram_tensor("qs", [SH, DIN], F32R, kind="ExternalInput")
    ks = nc.dram_tensor("ks", [SH, DIN], F32R, kind="ExternalInput")
    vt = nc.dram_tensor("vt", [DIN, SH], BF16, kind="ExternalInput")
    wkt = nc.dram_tensor("wkt", [DIN, DK], F32, kind="ExternalInput")
    wqts = nc.dram_tensor("wqts", [DIN, DQ], F32, kind="ExternalInput")
    wv = nc.dram_tensor("wv", [DK, DIN], BF16, kind="ExternalInput")
    onesbv = nc.dram_tensor("onesbv", [DK, 2], BF16, kind="ExternalInput")
    maskpt = nc.dram_tensor("maskpt", [DK, DQ], F32, kind="ExternalInput")
    out = nc.dram_tensor("out", [DQ, SH], F32, kind="ExternalOutput")

    n_s4 = SH // 512  # 8 iterations of 4 s-tiles

    with tile.TileContext(nc) as tc:
        with (
            tc.tile_pool(name="io", bufs=2) as io,
            tc.tile_pool(name="wpool", bufs=1) as wp,
            tc.tile_pool(name="work", bufs=1) as wk,
            tc.tile_pool(name="ostage", bufs=2) as ost,
            tc.tile_pool(name="ps", bufs=4, space="PSUM") as ps,
            tc.tile_pool(name="dram", bufs=1, space="DRAM") as dram,
        ):
            # ---------------- Phase A: Gt = k^T q (local Gram, fp32r) -------
            # Gt[j, i] = sum_s k[s, j] q[s, i]; 4 j-chunks in PSUM.
            gt_ps = [ps.tile([128, 512], F32, tag="psA", name=f"gt{j}") for j in range(4)]
            qs_r = qs[:, :].rearrange("(g c p) d -> g p c d", p=128, c=4)
            ks_r = ks[:, :].rearrange("(g c p) d -> g p c d", p=128, c=4)
            for g in range(n_s4):
                q4 = io.tile([128, 2048], F32R, tag="q4", name="q4")
                k4 = io.tile([128, 2048], F32R, tag="k4", name="k4")
                nc.sync.dma_start(out=q4[:, :].rearrange("p (c d) -> p c d", c=4), in_=qs_r[g])
                nc.sync.dma_start(out=k4[:, :].rearrange("p (c d) -> p c d", c=4), in_=ks_r[g])
                for c in range(4):
                    rhs = q4[:, c * 512 : (c + 1) * 512]
                    for j in range(4):
                        lhsT = k4[:, c * 512 + j * 128 : c * 512 + (j + 1) * 128]
                        nc.tensor.matmul(
                            gt_ps[j][:, :],
                            lhsT,
                            rhs,
                            start=(g == 0 and c == 0),
                            stop=(g == n_s4 - 1 and c == 3),
                        )

            # ---------------- Phase B: AllReduce Gt within batch pair -------
            gtsb = wk.tile([128, 2048], F32, tag="gtsb", name="gtsb")
            for j in range(4):
                nc.vector.tensor_copy(gtsb[:, j * 512 : (j + 1) * 512], gt_ps[j][:, :])
            garin = dram.tile([DIN, DIN], F32, tag="garin", name="garin")
            garout = dram.tile([DIN, DIN], F32, tag="garout", name="garout")
            nc.sync.dma_start(
                out=garin[:, :].rearrange("(c p) e -> p c e", p=128),
                in_=gtsb[:, :].rearrange("p (c e) -> p c e", c=4),
            )
            nc.gpsimd.collective_compute(
                "AllReduce",
                mybir.AluOpType.add,
                replica_groups=[[0, 1], [2, 3], [4, 5], [6, 7]],
                ins=[garin[:, :].opt()],
                outs=[garout[:, :].opt()],
            )

            # Prefetch weights + vT while the collective is in flight.
            wkt_sb = wp.tile([128, 2048], F32, tag="wkt", name="wkt_sb")
            nc.sync.dma_start(
                out=wkt_sb[:, :].rearrange("p (c e) -> p c e", c=4), in_=wkt[:, :].rearrange("(c p) e -> p c e", p=128)
            )
            wqts_sb = wp.tile([128, 2048], F32, tag="wqts", name="wqts_sb")
            nc.sync.dma_start(
                out=wqts_sb[:, :].rearrange("p (c a) -> p c a", c=4), in_=wqts[:, :].rearrange("(c p) a -> p c a", p=128)
            )
            wv_sb = wp.tile([128, 2048], BF16, tag="wv", name="wv_sb")
            nc.sync.dma_start(
                out=wv_sb[:, :].rearrange("p (c j) -> p c j", c=4), in_=wv[:, :].rearrange("(c p) j -> p c j", p=128)
            )
            onesbv_sb = wp.tile([128, 8], BF16, tag="onesbv", name="onesbv_sb")
            nc.sync.dma_start(
                out=onesbv_sb[:, :].rearrange("p (c t) -> p c t", c=4),
                in_=onesbv[:, :].rearrange("(c p) t -> p c t", p=128),
            )
            maskpt_sb = wp.tile([128, 2048], F32, tag="maskpt", name="maskpt_sb")
            nc.sync.dma_start(
                out=maskpt_sb[:, :].rearrange("p (c a) -> p c a", c=4),
                in_=maskpt[:, :].rearrange("(c p) a -> p c a", p=128),
            )
            vt_sb = []
            for jc in range(4):
                t = wp.tile([128, SH], BF16, tag=f"vt{jc}", name=f"vt{jc}")
                nc.sync.dma_start(out=t[:, :], in_=vt[jc * 128 : (jc + 1) * 128, :])
                vt_sb.append(t)

            gtfull = wk.tile([128, 2048], F32, tag="gtfull", name="gtfull")
            nc.sync.dma_start(
                out=gtfull[:, :].rearrange("p (c e) -> p c e", c=4),
                in_=garout[:, :].rearrange("(c p) e -> p c e", p=128),
            )

            # ---------------- Phase C: numT = (Wq G Wk^T)^T, scaled --------
            # T1[i, e] = sum_j Gt[j, i] WkT[j, e]
            t1_ps = [ps.tile([128, 512], F32, tag="psA", name=f"t1{i}") for i in range(4)]
            for ic in range(4):
                for jc in range(4):
                    lhsT = gtfull[:, jc * 512 + ic * 128 : jc * 512 + (ic + 1) * 128]
                    nc.tensor.matmul(
                        t1_ps[ic][:, :],
                        lhsT,
                        wkt_sb[:, jc * 512 : (jc + 1) * 512],
                        start=(jc == 0),
                        stop=(jc == 3),
                    )
            t1_sb = wk.tile([128, 2048], F32, tag="t1sb", name="t1_sb")
            for ic in range(4):
                nc.vector.tensor_copy(t1_sb[:, ic * 512 : (ic + 1) * 512], t1_ps[ic][:, :])

            # numT[e, a] = sum_i T1[i, e] WqTs[i, a]   (WqTs pre-scaled 1/sqrt(DK))
            et_sb = wk.tile([128, 2048], BF16, tag="et", name="et_sb")
            for ec in range(4):
                numt_ps = ps.tile([128, 512], F32, tag="psB", name="numt")
                for ic in range(4):
                    lhsT = t1_sb[:, ic * 512 + ec * 128 : ic * 512 + (ec + 1) * 128]
                    nc.tensor.matmul(
                        numt_ps[:, :],
                        lhsT,
                        wqts_sb[:, ic * 512 : (ic + 1) * 512],
                        start=(ic == 0),
                        stop=(ic == 3),
                    )
                sc = wk.tile([128, 512], F32, tag="sc", bufs=2, name="sc")
                nc.vector.tensor_add(
                    sc[:, :], numt_ps[:, :], maskpt_sb[:, ec * 512 : (ec + 1) * 512]
                )
                nc.scalar.activation(et_sb[:, ec * 512 : (ec + 1) * 512], sc[:, :], AF.Exp)

            # ---------------- Phase D: row sums + bv term ------------------
            # rsum_ps[ac][:,0] = rowsum(E) per dq, [:,1] = E @ bv
            rr_sb, bias_sb = [], []
            for ac in range(4):
                rs_ps = ps.tile([128, 2], F32, tag="psB", name=f"rs{ac}")
                for ec in range(4):
                    lhsT = et_sb[:, ec * 512 + ac * 128 : ec * 512 + (ac + 1) * 128]
                    nc.tensor.matmul(
                        rs_ps[:, :],
                        lhsT,
                        onesbv_sb[:, ec * 2 : (ec + 1) * 2],
                        start=(ec == 0),
                        stop=(ec == 3),
                    )
                rr = wk.tile([128, 1], F32, tag=f"rr{ac}", name=f"rr{ac}")
                nc.vector.reciprocal(rr[:, :], rs_ps[:, 0:1])
                bi = wk.tile([128, 1], F32, tag=f"bi{ac}", name=f"bi{ac}")
                nc.vector.tensor_mul(bi[:, :], rs_ps[:, 1:2], rr[:, :])
                rr_sb.append(rr)
                bias_sb.append(bi)

            # ---------------- Phase E: ApT = (E Wv)^T (bf16) ---------------
            apt_sb = wk.tile([128, 2048], BF16, tag="apt", name="apt_sb")
            for jc in range(4):
                apt_ps = ps.tile([128, 512], F32, tag="psA", name="aptps")
                for ec in range(4):
                    lhsT = wv_sb[:, ec * 512 + jc * 128 : ec * 512 + (jc + 1) * 128]
                    nc.tensor.matmul(
                        apt_ps[:, :],
                        lhsT,
                        et_sb[:, ec * 512 : (ec + 1) * 512],
                        start=(ec == 0),
                        stop=(ec == 3),
                    )
                nc.vector.tensor_copy(apt_sb[:, jc * 512 : (jc + 1) * 512], apt_ps[:, :])

            # ---------------- Phase F: out = rr * (Ap v^T) + rr*ebv --------
            for ac in range(4):
                for sg in range(2):  # two groups of 4 s-tiles (PSUM dbl-buffer)
                    tagz = "psB" if sg else "psA"
                    o_ps = [
                        ps.tile([128, 512], F32, tag=tagz, name=f"o{st}")
                        for st in range(4)
                    ]
                    for jc in range(4):
                        lhsT = apt_sb[:, jc * 512 + ac * 128 : jc * 512 + (ac + 1) * 128]
                        for st in range(4):
                            s0 = sg * 2048 + st * 512
                            nc.tensor.matmul(
                                o_ps[st][:, :],
                                lhsT,
                                vt_sb[jc][:, s0 : s0 + 512],
                                start=(jc == 0),
                                stop=(jc == 3),
                            )
                    o_sb = ost.tile([128, 2048], F32, tag="osb", name="o_sb")
                    for st in range(4):
                        nc.scalar.activation(
                            o_sb[:, st * 512 : (st + 1) * 512],
                            o_ps[st][:, :],
                            AF.Identity,
                            bias=bias_sb[ac][:, :],
                            scale=rr_sb[ac][:, :],
                        )
                    nc.sync.dma_start(
                        out=out[ac * 128 : (ac + 1) * 128, sg * 2048 : (sg + 1) * 2048],
                        in_=o_sb[:, :],
                    )

    _split_multi_waits(nc)
    return nc


def kernel(q, k, v, Wq, bq, Wk, bk, Wv, bv, global_tokens):
    global _NC_CACHE, LAST_RESULTS
    q = np.asarray(q, dtype=np.float32)
    k = np.asarray(k, dtype=np.float32)
    v = np.asarray(v, dtype=np.float32)
    Wq = np.asarray(Wq, dtype=np.float32)
    bq = np.asarray(bq, dtype=np.float32)
    Wk = np.asarray(Wk, dtype=np.float32)
    bk = np.asarray(bk, dtype=np.float32)
    Wv = np.asarray(Wv, dtype=np.float32)
    bv = np.asarray(bv, dtype=np.float32)
    gt_idx = np.asarray(global_tokens)

    # host: sparse-attention additive mask
    idx = np.arange(DK)
    glb = np.zeros(DK, dtype=bool)
    glb[gt_idx] = True
    cond = (idx[:, None] < idx[None, :]) & (~glb[:, None]) & (~glb[None, :])
    mask = np.where(cond, np.float32(MASK_NEG), np.float32(0.0)).astype(np.float32)

    # host: fold projection-bias rank-1 terms into the additive mask (per batch)
    scale = 1.0 / np.sqrt(DK)
    qsum = q.sum(axis=1)  # [B, DIN]
    ksum = k.sum(axis=1)  # [B, DIN]
    a_vec = qsum @ Wq.T  # [B, DQ]  (= Wq @ qsum_b)
    c_vec = ksum @ Wk.T  # [B, DK]
    maskpt_b = []
    for b in range(B):
        corr = (
            np.outer(a_vec[b], bk)
            + np.outer(bq, c_vec[b])
            + np.float32(S) * np.outer(bq, bk)
        )
        maskpt_b.append(((mask + corr) * scale).T.copy())  # [DK, DQ]

    wkt_h = np.ascontiguousarray(Wk.T)  # [DIN, DK]
    wqts_h = np.ascontiguousarray(Wq.T * scale).astype(np.float32)  # [DIN, DQ]
    wv_h = Wv.astype(ml_dtypes.bfloat16)  # [DK, DIN]
    onesbv_h = np.stack([np.ones(DK, np.float32), bv], axis=1).astype(
        ml_dtypes.bfloat16
    )  # [DK, 2]

    in_maps = []
    for c in range(N_CORES):
        b, h = c // 2, c % 2
        sl = slice(h * SH, (h + 1) * SH)
        in_maps.append(
            {
                "qs": np.ascontiguousarray(q[b, sl]),
                "ks": np.ascontiguousarray(k[b, sl]),
                "vt": np.ascontiguousarray(v[b, sl].T).astype(ml_dtypes.bfloat16),
                "wkt": wkt_h,
                "wqts": wqts_h,
                "wv": wv_h,
                "onesbv": onesbv_h,
                "maskpt": maskpt_b[b],
            }
        )

    if _NC_CACHE is None:
        _NC_CACHE = _build()
    res = run_bass_kernel_spmd(
        _NC_CACHE,
        in_maps,
        core_ids=list(range(N_CORES)),
        trace=TRACE,
        tmpdir=TRACE_DIR,
    )
    LAST_RESULTS = res

    out = np.empty((B, DQ, S), dtype=np.float32)
    for c in range(N_CORES):
        b, h = c // 2, c % 2
        out[b, :, h * SH : (h + 1) * SH] = res.results[c]["out"]
    return out


# revision 13
# speedup vs baseline: 1.3578x; 1.3578x over previous
"""Distributed sparse-attention head for Trainium2 (8 NeuronCores).

Math (per batch b):
    Q = q Wq^T + bq ; K = k Wk^T + bk ; V = v Wv^T + bv
    num = Q^T K  (contract over sequence S)
    attn = softmax((num + mask)/sqrt(DK), axis=-1)
    out = attn V^T                       # [DQ, S]

Key restructuring (avoids all big on-device transposes):
    num  = Wq G Wk^T + rank-1 bias terms,  G = q^T k   (natural [s,d] layout!)
    out  = diag(1/rowsum(E)) (E Wv) v^T + rank-1 bv term,  E = exp(scores)
The rank-1 bias corrections are folded into a host-precomputed additive
mask tensor. Softmax max-subtraction is skipped (scores bounded ~ +-30,
exp safe in fp32). Sharding: core c -> (batch b=c//2, seq-half h=c%2);
G is AllReduce'd (add) within the 2-core pair that shares a batch.
"""

import sys

sys.path.insert(0, "/opt/trn_rl_repo")

import numpy as np
import ml_dtypes
import concourse.bass as bass
import concourse.mybir as mybir
import concourse.tile as tile
from concourse.bass_utils import run_bass_kernel_spmd
from concourse.vector_clock import ScopedClock

B, S, DIN, DQ, DK = 4, 8192, 512, 512, 512
SH = S // 2  # 4096 seq positions per core
N_CORES = 8
F32 = mybir.dt.float32
F32R = mybir.dt.float32r
BF16 = mybir.dt.bfloat16
AF = mybir.ActivationFunctionType

# mask value (post-scale): exp(-200) == 0 in fp32, comfortably beyond any score
MASK_NEG = -200.0 * np.sqrt(DK)

TRACE = False
TRACE_DIR = None
LAST_RESULTS = None


def _patched_drain_and_barrier(self, tick_clock, wait_clock):
    # This walrus build rejects >1 sync-wait on the kernel-tail Drain
    # ("Too many sync wait commands"). Put the global-clock waits on
    # no-fuse NOPs (one wait each), then emit a clean drain.
    nc = self.nc
    probe = nc.sync.nop(nofuse=True)
    wait_clock.add_sem_waits(probe.ins, ScopedClock({None: tick_clock.global_clock}))
    waits = list(probe.ins.sync_info.on_wait)
    probe.ins.sync_info.on_wait[:] = waits[:1]
    for w in waits[1:]:
        n2 = nc.sync.nop(nofuse=True)
        if n2.ins.sync_info is None:
            n2.ins.sync_info = mybir.SyncInfo(on_wait=[w], on_update=[])
        else:
            n2.ins.sync_info.on_wait[:] = [w]
    nc.sync.drain()
    nc.all_engine_barrier()
    assert self.sems is not None
    popped = nc._tile_sem_poison_stack.pop()
    assert popped is self._sem_poison
    nc.clear_and_free_semaphores(list(self.sems.allocated().values()))
    nc.all_engine_barrier()


tile.TileContext._drain_and_barrier = _patched_drain_and_barrier


def _split_multi_waits(nc, max_waits=1):
    """This walrus build rejects instructions carrying more than one sync
    wait ("Too many sync wait commands"). Hoist extra waits onto NoOp
    instructions spliced immediately before the carrier, same engine —
    semantically identical (engine blocks on the waits either way)."""
    uid = 0
    for fn in nc.m.functions:
        for bb in fn.blocks:
            new_insts = []
            for ins in bb.instructions:
                si = ins.sync_info
                if si is not None and len(si.on_wait) > max_waits:
                    extra = si.on_wait[: len(si.on_wait) - max_waits]
                    keep = si.on_wait[len(si.on_wait) - max_waits :]
                    for w in extra:
                        uid += 1
                        nop = mybir.InstNoOp(
                            name=f"{ins.name}-wsplit{uid}",
                            ins=[],
                            outs=[],
                        )
                        nop.engine = ins.engine
                        nop.sync_info = mybir.SyncInfo(on_wait=[w], on_update=[])
                        nop.bass_nofuse = True
                        new_insts.append(nop)
                    si.on_wait[:] = keep
                new_insts.append(ins)
            bb.instructions[:] = new_insts


_NC_CACHE = None


def _build():
    """Build the SPMD program (identical on all 8 cores)."""
    nc = bass.Bass(target_bir_lowering=False)

    qs = nc.dram_tensor("qs", [SH, DIN], F32R, kind="ExternalInput")
    ks = nc.dram_tensor("ks", [SH, DIN], F32R, kind="ExternalInput")
    vt = nc.dram_tensor("vt", [DIN, SH], BF16, kind="ExternalInput")
    wkt = nc.dram_tensor("wkt", [DIN, DK], BF16, kind="ExternalInput")
    wqts = nc.dram_tensor("wqts", [DIN, DQ], F32R, kind="ExternalInput")
    wv = nc.dram_tensor("wv", [DK, DIN], BF16, kind="ExternalInput")
    onesbv = nc.dram_tensor("onesbv", [DK, 2], BF16, kind="ExternalInput")
    maskpt = nc.dram_tensor("maskpt", [DK, DQ], F32, kind="ExternalInput")
    out = nc.dram_tensor("out", [DQ, SH], F32, kind="ExternalOutput")

    n_s4 = SH // 512  # 8 iterations of 4 s-tiles
    MUL, ADD = mybir.AluOpType.mult, mybir.AluOpType.add

    with tile.TileContext(nc) as tc:
        with (
            tc.tile_pool(name="io", bufs=3) as io,
            tc.tile_pool(name="wpool", bufs=1) as wp,
            tc.tile_pool(name="work", bufs=1) as wk,
            tc.tile_pool(name="ostage", bufs=3) as ost,
            tc.tile_pool(name="ps", bufs=4, space="PSUM") as ps,
            tc.tile_pool(name="dram", bufs=1, space="DRAM") as dram,
        ):
            # ---- Phase A: Gt = k^T q (local Gram, fp32r), split in two s-chunks
            # so the first AllReduce overlaps the second half's matmuls.
            qs_r = qs[:, :].rearrange("(g c p) d -> g p c d", p=128, c=4)
            ks_r = ks[:, :].rearrange("(g c p) d -> g p c d", p=128, c=4)
            gt_ps = {}
            gtsb = {}
            garin = {}
            garout = {}

            def gram_chunk(h, g_lo, g_hi, tag):
                gt_ps[h] = [
                    ps.tile([128, 512], F32, tag=tag, name=f"gt{h}_{j}")
                    for j in range(4)
                ]
                for g in range(g_lo, g_hi):
                    q4 = io.tile([128, 2048], F32R, tag="q4", name="q4")
                    k4 = io.tile([128, 2048], F32R, tag="k4", name="k4")
                    nc.sync.dma_start(
                        out=q4[:, :].rearrange("p (c d) -> p c d", c=4), in_=qs_r[g]
                    )
                    nc.sync.dma_start(
                        out=k4[:, :].rearrange("p (c d) -> p c d", c=4), in_=ks_r[g]
                    )
                    for c in range(4):
                        rhs = q4[:, c * 512 : (c + 1) * 512]
                        for j in range(4):
                            lhsT = k4[:, c * 512 + j * 128 : c * 512 + (j + 1) * 128]
                            nc.tensor.matmul(
                                gt_ps[h][j][:, :],
                                lhsT,
                                rhs,
                                start=(g == g_lo and c == 0),
                                stop=(g == g_hi - 1 and c == 3),
                            )

            def gram_reduce(h):
                # PSUM -> bf16 SBUF -> DRAM bounce -> pairwise AllReduce
                gtsb[h] = wk.tile([128, 2048], BF16, tag=f"gtsb{h}", name=f"gtsb{h}")
                for j in range(4):
                    nc.vector.tensor_copy(
                        gtsb[h][:, j * 512 : (j + 1) * 512], gt_ps[h][j][:, :]
                    )
                garin[h] = dram.tile([DIN, DIN], BF16, tag=f"gari{h}", name=f"gari{h}")
                garout[h] = dram.tile([DIN, DIN], BF16, tag=f"garo{h}", name=f"garo{h}")
                nc.sync.dma_start(
                    out=garin[h][:, :].rearrange("(c p) e -> p c e", p=128),
                    in_=gtsb[h][:, :].rearrange("p (c e) -> p c e", c=4),
                )
                nc.gpsimd.collective_compute(
                    "AllReduce",
                    mybir.AluOpType.add,
                    replica_groups=[[0, 1], [2, 3], [4, 5], [6, 7]],
                    ins=[garin[h][:, :].opt()],
                    outs=[garout[h][:, :].opt()],
                )

            gram_chunk(0, 0, n_s4 // 2, "psA")
            gram_reduce(0)  # AR1 flies while chunk 1 computes
            gram_chunk(1, n_s4 // 2, n_s4, "psB")
            gram_reduce(1)

            # Prefetch weights + vT (queued behind phase-A loads; fill AR window).
            wkt_sb = wp.tile([128, 2048], BF16, tag="wkt", name="wkt_sb")
            nc.sync.dma_start(
                out=wkt_sb[:, :].rearrange("p (c e) -> p c e", c=4),
                in_=wkt[:, :].rearrange("(c p) e -> p c e", p=128),
            )
            wqts_sb = wp.tile([128, 2048], F32R, tag="wqts", name="wqts_sb")
            nc.sync.dma_start(
                out=wqts_sb[:, :].rearrange("p (c a) -> p c a", c=4),
                in_=wqts[:, :].rearrange("(c p) a -> p c a", p=128),
            )
            wv_sb = wp.tile([128, 2048], BF16, tag="wv", name="wv_sb")
            nc.sync.dma_start(
                out=wv_sb[:, :].rearrange("p (c j) -> p c j", c=4),
                in_=wv[:, :].rearrange("(c p) j -> p c j", p=128),
            )
            onesbv_sb = wp.tile([128, 8], BF16, tag="onesbv", name="onesbv_sb")
            nc.sync.dma_start(
                out=onesbv_sb[:, :].rearrange("p (c t) -> p c t", c=4),
                in_=onesbv[:, :].rearrange("(c p) t -> p c t", p=128),
            )
            maskpt_sb = wp.tile([128, 2048], F32, tag="maskpt", name="maskpt_sb")
            nc.sync.dma_start(
                out=maskpt_sb[:, :].rearrange("p (c a) -> p c a", c=4),
                in_=maskpt[:, :].rearrange("(c p) a -> p c a", p=128),
            )
            vt_sb = []
            for jc in range(4):
                t = wp.tile([128, SH], BF16, tag=f"vt{jc}", name=f"vt{jc}")
                nc.sync.dma_start(out=t[:, :], in_=vt[jc * 128 : (jc + 1) * 128, :])
                vt_sb.append(t)

            # ---- Phase C: T1 = G Wk^T accumulated over the two AR chunks ----
            # T1[i, e] = sum_j Gt[j, i] WkT[j, e]
            gtfull = {}
            t1_ps = [ps.tile([128, 512], F32, tag="psA", name=f"t1{i}") for i in range(4)]
            for h in range(2):
                gtfull[h] = wk.tile([128, 2048], BF16, tag=f"gtf{h}", name=f"gtf{h}")
                nc.sync.dma_start(
                    out=gtfull[h][:, :].rearrange("p (c e) -> p c e", c=4),
                    in_=garout[h][:, :].rearrange("(c p) e -> p c e", p=128),
                )
                for ic in range(4):
                    for jc in range(4):
                        lhsT = gtfull[h][
                            :, jc * 512 + ic * 128 : jc * 512 + (ic + 1) * 128
                        ]
                        nc.tensor.matmul(
                            t1_ps[ic][:, :],
                            lhsT,
                            wkt_sb[:, jc * 512 : (jc + 1) * 512],
                            start=(h == 0 and jc == 0),
                            stop=(h == 1 and jc == 3),
                        )
            t1_sb = wk.tile([128, 2048], F32R, tag="t1sb", name="t1_sb")
            for ic in range(4):
                nc.vector.tensor_copy(t1_sb[:, ic * 512 : (ic + 1) * 512], t1_ps[ic][:, :])

            # numT[e, a] = sum_i T1[i, e] WqTs[i, a]   (WqTs pre-scaled 1/sqrt(DK))
            et_sb = wk.tile([128, 2048], BF16, tag="et", name="et_sb")
            for ec in range(4):
                numt_ps = ps.tile([128, 512], F32, tag="psB", name="numt")
                for ic in range(4):
                    lhsT = t1_sb[:, ic * 512 + ec * 128 : ic * 512 + (ec + 1) * 128]
                    nc.tensor.matmul(
                        numt_ps[:, :],
                        lhsT,
                        wqts_sb[:, ic * 512 : (ic + 1) * 512],
                        start=(ic == 0),
                        stop=(ic == 3),
                    )
                sc = wk.tile([128, 512], F32, tag="sc", bufs=2, name="sc")
                nc.vector.tensor_add(
                    sc[:, :], numt_ps[:, :], maskpt_sb[:, ec * 512 : (ec + 1) * 512]
                )
                nc.scalar.activation(et_sb[:, ec * 512 : (ec + 1) * 512], sc[:, :], AF.Exp)

            # ---- Phase D: row sums + bv term -------------------------------
            # rs_ps[:,0] = rowsum(E) per dq, [:,1] = E @ bv
            rr_sb, bias_sb = [], []
            for ac in range(4):
                rs_ps = ps.tile([128, 2], F32, tag="psA", name=f"rs{ac}")
                for ec in range(4):
                    lhsT = et_sb[:, ec * 512 + ac * 128 : ec * 512 + (ac + 1) * 128]
                    nc.tensor.matmul(
                        rs_ps[:, :],
                        lhsT,
                        onesbv_sb[:, ec * 2 : (ec + 1) * 2],
                        start=(ec == 0),
                        stop=(ec == 3),
                    )
                rr = wk.tile([128, 1], F32, tag=f"rr{ac}", name=f"rr{ac}")
                nc.vector.reciprocal(rr[:, :], rs_ps[:, 0:1])
                bi = wk.tile([128, 1], F32, tag=f"bi{ac}", name=f"bi{ac}")
                nc.vector.tensor_mul(bi[:, :], rs_ps[:, 1:2], rr[:, :])
                rr_sb.append(rr)
                bias_sb.append(bi)

            # ---- Phase E: ApT = (E Wv)^T (bf16) ----------------------------
            apt_sb = wk.tile([128, 2048], BF16, tag="apt", name="apt_sb")
            for jc in range(4):
                apt_ps = ps.tile([128, 512], F32, tag="psB", name="aptps")
                for ec in range(4):
                    lhsT = wv_sb[:, ec * 512 + jc * 128 : ec * 512 + (jc + 1) * 128]
                    nc.tensor.matmul(
                        apt_ps[:, :],
                        lhsT,
                        et_sb[:, ec * 512 : (ec + 1) * 512],
                        start=(ec == 0),
                        stop=(ec == 3),
                    )
                nc.vector.tensor_copy(apt_sb[:, jc * 512 : (jc + 1) * 512], apt_ps[:, :])

            # ---- Phase F: out = rr * (Ap v^T) + rr*ebv ---------------------
            for ac in range(4):
                for sg in range(2):  # two groups of 4 s-tiles (PSUM dbl-buffer)
                    tagz = "psB" if sg else "psA"
                    o_ps = [
                        ps.tile([128, 512], F32, tag=tagz, name=f"o{st}")
                        for st in range(4)
                    ]
                    for jc in range(4):
                        lhsT = apt_sb[:, jc * 512 + ac * 128 : jc * 512 + (ac + 1) * 128]
                        for st in range(4):
                            s0 = sg * 2048 + st * 512
                            nc.tensor.matmul(
                                o_ps[st][:, :],
                                lhsT,
                                vt_sb[jc][:, s0 : s0 + 512],
                                start=(jc == 0),
                                stop=(jc == 3),
                            )
                    o_sb = ost.tile([128, 2048], F32, tag="osb", name="o_sb")
                    for st in range(4):
                        # split evictions across ACT and DVE so neither gates PE
                        if st % 2 == 0:
                            nc.scalar.activation(
                                o_sb[:, st * 512 : (st + 1) * 512],
                                o_ps[st][:, :],
                                AF.Identity,
                                bias=bias_sb[ac][:, :],
                                scale=rr_sb[ac][:, :],
                            )
                        else:
                            nc.vector.tensor_scalar(
                                o_sb[:, st * 512 : (st + 1) * 512],
                                o_ps[st][:, :],
                                rr_sb[ac][:, :],
                                bias_sb[ac][:, :],
                                MUL,
                                ADD,
                            )
                    nc.sync.dma_start(
                        out=out[ac * 128 : (ac + 1) * 128, sg * 2048 : (sg + 1) * 2048],
                        in_=o_sb[:, :],
                    )

    _split_multi_waits(nc)
    return nc


def kernel(q, k, v, Wq, bq, Wk, bk, Wv, bv, global_tokens):
    global _NC_CACHE, LAST_RESULTS
    q = np.asarray(q, dtype=np.float32)
    k = np.asarray(k, dtype=np.float32)
    v = np.asarray(v, dtype=np.float32)
    Wq = np.asarray(Wq, dtype=np.float32)
    bq = np.asarray(bq, dtype=np.float32)
    Wk = np.asarray(Wk, dtype=np.float32)
    bk = np.asarray(bk, dtype=np.float32)
    Wv = np.asarray(Wv, dtype=np.float32)
    bv = np.asarray(bv, dtype=np.float32)
    gt_idx = np.asarray(global_tokens)

    # host: sparse-attention additive mask
    idx = np.arange(DK)
    glb = np.zeros(DK, dtype=bool)
    glb[gt_idx] = True
    cond = (idx[:, None] < idx[None, :]) & (~glb[:, None]) & (~glb[None, :])
    mask = np.where(cond, np.float32(MASK_NEG), np.float32(0.0)).astype(np.float32)

    # host: fold projection-bias rank-1 terms into the additive mask (per batch)
    scale = 1.0 / np.sqrt(DK)
    qsum = q.sum(axis=1)  # [B, DIN]
    ksum = k.sum(axis=1)  # [B, DIN]
    a_vec = qsum @ Wq.T  # [B, DQ]  (= Wq @ qsum_b)
    c_vec = ksum @ Wk.T  # [B, DK]
    maskpt_b = []
    for b in range(B):
        corr = (
            np.outer(a_vec[b], bk)
            + np.outer(bq, c_vec[b])
            + np.float32(S) * np.outer(bq, bk)
        )
        maskpt_b.append(((mask + corr) * scale).T.copy())  # [DK, DQ]

    wkt_h = np.ascontiguousarray(Wk.T).astype(ml_dtypes.bfloat16)  # [DIN, DK]
    wqts_h = np.ascontiguousarray(Wq.T * scale).astype(np.float32)  # [DIN, DQ]
    wv_h = Wv.astype(ml_dtypes.bfloat16)  # [DK, DIN]
    onesbv_h = np.stack([np.ones(DK, np.float32), bv], axis=1).astype(
        ml_dtypes.bfloat16
    )  # [DK, 2]

    in_maps = []
    for c in range(N_CORES):
        b, h = c // 2, c % 2
        sl = slice(h * SH, (h + 1) * SH)
        in_maps.append(
            {
                "qs": np.ascontiguousarray(q[b, sl]),
                "ks": np.ascontiguousarray(k[b, sl]),
                "vt": np.ascontiguousarray(v[b, sl].T).astype(ml_dtypes.bfloat16),
                "wkt": wkt_h,
                "wqts": wqts_h,
                "wv": wv_h,
                "onesbv": onesbv_h,
                "maskpt": maskpt_b[b],
            }
        )

    if _NC_CACHE is None:
        _NC_CACHE = _build()
    res = run_bass_kernel_spmd(
        _NC_CACHE,
        in_maps,
        core_ids=list(range(N_CORES)),
        trace=TRACE,
        tmpdir=TRACE_DIR,
    )
    LAST_RESULTS = res

    out = np.empty((B, DQ, S), dtype=np.float32)
    for c in range(N_CORES):
        b, h = c // 2, c % 2
        out[b, :, h * SH : (h + 1) * SH] = res.results[c]["out"]
    return out
